# revision 6
# baseline (speedup 1.0000x reference)
"""AttnDecoderRNN step on 8 Trainium2 NeuronCores.

Sharding: out_W column-parallel (vocab) across 8 cores; tiny attention+GRU
replicated on every core; embedding row gathered on host (4KB of emb_W);
log_softmax normalizer combined on host from per-core sum(exp(logits)).
"""

import numpy as np
from contextlib import ExitStack

import ml_dtypes

import concourse.bass as bass
import concourse.mybir as mybir
import concourse.tile as tile
from concourse import bacc
from concourse.bass_utils import run_bass_kernel_spmd

H = 1024
V = 50257
L = 15          # MAX_LEN
P = 128
KH = H // P     # 8 k-tiles per H-vector
K2 = 2 * H // P # 16 k-tiles per 2H-vector
M3 = 3 * H // P # 24 m-blocks of the 3H gate dim
NCORES = 8
CHUNK = 512

AF = mybir.ActivationFunctionType
F32 = mybir.dt.float32
PAD_BIAS = -10000.0  # exp() underflows to exactly 0.0


def _dt(name):
    return {"float32": F32, "bfloat16": mybir.dt.bfloat16}[name]


def _np_dt(name):
    return {"float32": np.float32, "bfloat16": ml_dtypes.bfloat16}[name]


def build_kernel(w_dtype="bfloat16", nch=13, num_cores=NCORES):
    """Build + compile the SPMD bass program. Returns compiled nc."""
    wdt = _dt(w_dtype)
    vsh = nch * CHUNK  # padded vocab shard per core

    nc = bacc.Bacc(
        "TRN2",
        target_bir_lowering=False,
        debug=False,
        enable_asserts=True,
        num_devices=num_cores,
    )

    # ---- I/O ----
    emb_kt = nc.dram_tensor("emb_kt", [P, KH], F32, kind="ExternalInput").ap()
    h0_kt = nc.dram_tensor("h0_kt", [P, KH], F32, kind="ExternalInput").ap()
    enc = nc.dram_tensor("enc", [L, H], F32, kind="ExternalInput").ap()
    attn_WT = nc.dram_tensor("attn_WT", [P, K2, L], F32, kind="ExternalInput").ap()
    attn_bT = nc.dram_tensor("attn_bT", [L, 1], F32, kind="ExternalInput").ap()
    attn_b = nc.dram_tensor("attn_b", [1, L], F32, kind="ExternalInput").ap()
    comb_WT = nc.dram_tensor("comb_WT", [2 * H, H], wdt, kind="ExternalInput").ap()
    w_ihT = nc.dram_tensor("w_ihT", [H, 3 * H], wdt, kind="ExternalInput").ap()
    w_hhT = nc.dram_tensor("w_hhT", [H, 3 * H], wdt, kind="ExternalInput").ap()
    b_comb = nc.dram_tensor("b_comb", [P, KH], F32, kind="ExternalInput").ap()
    b_r = nc.dram_tensor("b_r", [P, KH], F32, kind="ExternalInput").ap()
    b_z = nc.dram_tensor("b_z", [P, KH], F32, kind="ExternalInput").ap()
    b_in = nc.dram_tensor("b_in", [P, KH], F32, kind="ExternalInput").ap()
    b_hn = nc.dram_tensor("b_hn", [P, KH], F32, kind="ExternalInput").ap()
    outWT = nc.dram_tensor("outWT", [H, vsh], wdt, kind="ExternalInput").ap()
    outb = nc.dram_tensor("outb", [1, vsh], F32, kind="ExternalInput").ap()

    logits_s = nc.dram_tensor("logits_s", [1, vsh], F32, kind="ExternalOutput").ap()
    sumexp = nc.dram_tensor("sumexp", [1, 1], F32, kind="ExternalOutput").ap()
    h_new_kt = nc.dram_tensor("h_new_kt", [P, KH], F32, kind="ExternalOutput").ap()
    attn_w_out = nc.dram_tensor("attn_w", [1, L], F32, kind="ExternalOutput").ap()

    with tile.TileContext(nc) as tc:
        with ExitStack() as ctx:
            _body(ctx, tc, wdt, nch, vsh, locals())

    nc.compile()
    return nc


def _body(ctx, tc, wdt, nch, vsh, t):
    nc = tc.nc
    fp32 = wdt == F32

    consts = ctx.enter_context(tc.tile_pool(name="consts", bufs=1))
    cpool = ctx.enter_context(tc.tile_pool(name="cpool", bufs=K2))
    wpool = ctx.enter_context(tc.tile_pool(name="wpool", bufs=KH))
    opool = ctx.enter_context(tc.tile_pool(name="opool", bufs=12))
    pp_s1 = ctx.enter_context(
        tc.tile_pool(name="pp_s1", bufs=2, space=bass.MemorySpace.PSUM))
    pp_s2 = ctx.enter_context(
        tc.tile_pool(name="pp_s2", bufs=1, space=bass.MemorySpace.PSUM))
    pp_gates = ctx.enter_context(
        tc.tile_pool(name="pp_gates", bufs=1, space=bass.MemorySpace.PSUM))
    pp_log = ctx.enter_context(
        tc.tile_pool(name="pp_log", bufs=3, space=bass.MemorySpace.PSUM))

    # ---- constant / small loads ----
    emb_t = consts.tile([P, KH], F32)
    nc.sync.dma_start(out=emb_t[:], in_=t["emb_kt"])
    h0_t = consts.tile([P, KH], F32)
    nc.sync.dma_start(out=h0_t[:], in_=t["h0_kt"])
    enc_t = consts.tile([L, H], F32)
    nc.sync.dma_start(out=enc_t[:], in_=t["enc"])
    attnW_t = consts.tile([P, K2, L], F32)
    nc.sync.dma_start(out=attnW_t[:], in_=t["attn_WT"])
    attn_bT_t = consts.tile([L, 1], F32)
    nc.sync.dma_start(out=attn_bT_t[:], in_=t["attn_bT"])
    attn_b_t = consts.tile([1, L], F32)
    nc.sync.dma_start(out=attn_b_t[:], in_=t["attn_b"])
    b_comb_t = consts.tile([P, KH], F32)
    nc.sync.dma_start(out=b_comb_t[:], in_=t["b_comb"])
    b_r_t = consts.tile([P, KH], F32)
    nc.sync.dma_start(out=b_r_t[:], in_=t["b_r"])
    b_z_t = consts.tile([P, KH], F32)
    nc.sync.dma_start(out=b_z_t[:], in_=t["b_z"])
    b_in_t = consts.tile([P, KH], F32)
    nc.sync.dma_start(out=b_in_t[:], in_=t["b_in"])
    b_hn_t = consts.tile([P, KH], F32)
    nc.sync.dma_start(out=b_hn_t[:], in_=t["b_hn"])
    outb_t = consts.tile([1, vsh], F32)
    nc.sync.dma_start(out=outb_t[:], in_=t["outb"])

    ones_t = consts.tile([1, P], F32)
    nc.vector.memset(ones_t[:], 1.0)

    attn_in = consts.tile([P, K2], F32)
    nc.sync.dma_start(out=attn_in[:, 0:KH], in_=t["emb_kt"])
    nc.sync.dma_start(out=attn_in[:, KH:K2], in_=t["h0_kt"])
    comb_in = consts.tile([P, K2], F32)
    nc.sync.dma_start(out=comb_in[:, 0:KH], in_=t["emb_kt"])

    # ---- attention scores (both layouts) ----
    scores_ps = pp_s1.tile([1, L], F32, tag="s1")
    for k in range(K2):
        nc.tensor.matmul(scores_ps[:], attn_in[:, k:k + 1], attnW_t[:, k, :],
                         start=(k == 0), stop=(k == K2 - 1))
    scoresT_ps = pp_s2.tile([L, 1], F32, tag="s2")
    for k in range(K2):
        nc.tensor.matmul(scoresT_ps[:], attnW_t[:, k, :], attn_in[:, k:k + 1],
                         start=(k == 0), stop=(k == K2 - 1))

    # attn_weights output = softmax(scores + b); no max-subtraction (|s| ~ 1)
    scores_sb = consts.tile([1, L], F32)
    nc.vector.tensor_add(scores_sb[:], scores_ps[:], attn_b_t[:])
    exp_sb = consts.tile([1, L], F32)
    se_sb = consts.tile([1, 1], F32)
    nc.scalar.activation(exp_sb[:], scores_sb[:], AF.Exp, accum_out=se_sb[:])
    inv_se = consts.tile([1, 1], F32)
    nc.vector.reciprocal(inv_se[:], se_sb[:])
    attn_w_sb = consts.tile([1, L], F32)
    nc.vector.tensor_scalar_mul(attn_w_sb[:], exp_sb[:], inv_se[:])
    nc.sync.dma_start(out=t["attn_w_out"], in_=attn_w_sb[:])

    # transposed exp(scores) on 15 partitions for the applied matmul
    expT_sb = consts.tile([L, 1], F32)
    nc.scalar.activation(expT_sb[:], scoresT_ps[:], AF.Exp, bias=attn_bT_t[:])

    # attn_applied^T (unnormalized): [128, 8] blocks = enc^T @ expT
    aa_ps = pp_s1.tile([P, KH], F32, tag="s1")
    for m in range(KH):
        nc.tensor.matmul(aa_ps[:, m:m + 1], enc_t[:, m * P:(m + 1) * P],
                         expT_sb[:], start=True, stop=True)

    # broadcast sum(exp) to 128 partitions: ones^T @ se
    bc_ps = pp_s2.tile([P, 1], F32, tag="s2")
    nc.tensor.matmul(bc_ps[:], ones_t[:], se_sb[:], start=True, stop=True)
    inv_bc = consts.tile([P, 1], F32)
    nc.vector.reciprocal(inv_bc[:], bc_ps[:])
    nc.vector.tensor_scalar_mul(comb_in[:, KH:K2], aa_ps[:], inv_bc[:])

    comb_in_mm = comb_in
    if not fp32:
        comb_in_mm = consts.tile([P, K2], wdt)
        nc.vector.tensor_copy(comb_in_mm[:], comb_in[:])

    # ---- combine layer: x^T = comb_WT^T-blocks @ comb_in cols ----
    # m-outer / k-inner: each psum column's accumulation group completes
    # before the next starts (start=True re-arms the whole 2KB psum bank).
    x_ps = pp_s1.tile([P, KH], F32, tag="s1")
    cws = []
    for k in range(K2):
        cw = cpool.tile([P, H], wdt, tag="cw")
        nc.sync.dma_start(out=cw[:], in_=t["comb_WT"][k * P:(k + 1) * P, :])
        cws.append(cw)
    for m in range(KH):
        for k in range(K2):
            nc.tensor.matmul(x_ps[:, m:m + 1], cws[k][:, m * P:(m + 1) * P],
                             comb_in_mm[:, k:k + 1],
                             start=(k == 0), stop=(k == K2 - 1))
    x_t = consts.tile([P, KH], F32)
    nc.vector.tensor_add(x_t[:], x_ps[:], b_comb_t[:])
    nc.vector.tensor_scalar_max(x_t[:], x_t[:], 0.0)

    x_mm, h0_mm = x_t, h0_t
    if not fp32:
        x_mm = consts.tile([P, KH], wdt)
        nc.vector.tensor_copy(x_mm[:], x_t[:])
        h0_mm = consts.tile([P, KH], wdt)
        nc.vector.tensor_copy(h0_mm[:], h0_t[:])

    # ---- GRU gates: gi^T, gh^T as [128, 24] psum ----
    MC = 12          # m-blocks per chunk
    WCH = MC * P     # 1536 columns per resident chunk
    gi_ps = pp_gates.tile([P, M3], F32, tag="gi")
    gh_ps = pp_gates.tile([P, M3], F32, tag="gh")
    for src, vec, g_ps in ((t["w_ihT"], x_mm, gi_ps), (t["w_hhT"], h0_mm, gh_ps)):
        for mc in range(3 * H // WCH):
            wts = []
            for k in range(KH):
                wt = wpool.tile([P, WCH], wdt, tag="wt")
                nc.sync.dma_start(
                    out=wt[:],
                    in_=src[k * P:(k + 1) * P, mc * WCH:(mc + 1) * WCH])
                wts.append(wt)
            for m in range(MC):
                gm = mc * MC + m
                for k in range(KH):
                    nc.tensor.matmul(g_ps[:, gm:gm + 1],
                                     wts[k][:, m * P:(m + 1) * P],
                                     vec[:, k:k + 1],
                                     start=(k == 0), stop=(k == KH - 1))

    # ---- gates elementwise on [128, 8] ----
    # (VectorE may read at most one PSUM operand -> stage gi in SBUF)
    gi_sb = consts.tile([P, M3], F32)
    nc.scalar.activation(gi_sb[:], gi_ps[:], AF.Copy)
    r_t = consts.tile([P, KH], F32)
    nc.vector.tensor_add(r_t[:], gi_sb[:, 0:KH], gh_ps[:, 0:KH])
    nc.vector.tensor_add(r_t[:], r_t[:], b_r_t[:])
    nc.scalar.activation(r_t[:], r_t[:], AF.Sigmoid)
    z_t = consts.tile([P, KH], F32)
    nc.vector.tensor_add(z_t[:], gi_sb[:, KH:2 * KH], gh_ps[:, KH:2 * KH])
    nc.vector.tensor_add(z_t[:], z_t[:], b_z_t[:])
    nc.scalar.activation(z_t[:], z_t[:], AF.Sigmoid)
    hn_t = consts.tile([P, KH], F32)
    nc.vector.tensor_add(hn_t[:], gh_ps[:, 2 * KH:3 * KH], b_hn_t[:])
    n_t = consts.tile([P, KH], F32)
    nc.vector.tensor_add(n_t[:], gi_sb[:, 2 * KH:3 * KH], b_in_t[:])
    nc.vector.tensor_mul(hn_t[:], r_t[:], hn_t[:])
    nc.vector.tensor_add(n_t[:], n_t[:], hn_t[:])
    nc.scalar.activation(n_t[:], n_t[:], AF.Tanh)
    # h_new = n + z * (h0 - n)
    d_t = consts.tile([P, KH], F32)
    nc.vector.tensor_sub(d_t[:], h0_t[:], n_t[:])
    h_t = consts.tile([P, KH], F32)
    nc.vector.tensor_mul(h_t[:], z_t[:], d_t[:])
    nc.vector.tensor_add(h_t[:], h_t[:], n_t[:])
    nc.sync.dma_start(out=t["h_new_kt"], in_=h_t[:])

    h_mm = h_t
    if not fp32:
        h_mm = consts.tile([P, KH], wdt)
        nc.vector.tensor_copy(h_mm[:], h_t[:])

    # ---- output projection: stream out_WT shard ----
    logits_sb = consts.tile([1, vsh], F32)
    se_slots = consts.tile([1, nch], F32)
    exp_tmp = consts.tile([1, CHUNK], F32)
    for c in range(nch):
        lg_ps = pp_log.tile([1, CHUNK], F32, tag="lg")
        for k in range(KH):
            ot = opool.tile([P, CHUNK], wdt, tag="ot")
            nc.sync.dma_start(
                out=ot[:],
                in_=t["outWT"][k * P:(k + 1) * P, c * CHUNK:(c + 1) * CHUNK])
            nc.tensor.matmul(lg_ps[:], h_mm[:, k:k + 1], ot[:],
                             start=(k == 0), stop=(k == KH - 1))
        sl = slice(c * CHUNK, (c + 1) * CHUNK)
        nc.vector.tensor_add(logits_sb[:, sl], lg_ps[:], outb_t[:, sl])
        nc.scalar.activation(exp_tmp[:], logits_sb[:, sl], AF.Exp,
                             accum_out=se_slots[:, c:c + 1])
    se_total = consts.tile([1, 1], F32)
    nc.vector.reduce_sum(se_total[:], se_slots[:], axis=mybir.AxisListType.X)
    nc.sync.dma_start(out=t["sumexp"], in_=se_total[:])
    nc.sync.dma_start(out=t["logits_s"], in_=logits_sb[:])


# ---------------------------------------------------------------------------
# host side
# ---------------------------------------------------------------------------

_CACHE = {}


def _get_nc(w_dtype, nch):
    key = (w_dtype, nch)
    if key not in _CACHE:
        _CACHE[key] = build_kernel(w_dtype, nch)
    return _CACHE[key]


def _kt(v):
    """[H*k] vector -> [128, k] partition-major tile layout."""
    v = np.asarray(v, np.float32)
    return np.ascontiguousarray(v.reshape(-1, P).T)


def make_in_maps(inputs, w_dtype="bfloat16", nch=13):
    wnp = _np_dt(w_dtype)
    vsh = nch * CHUNK
    vpad = vsh * NCORES

    idx = int(np.asarray(inputs["input_seq"]).reshape(-1)[0])
    emb_row = np.asarray(inputs["emb_W"], np.float32)[idx]
    h0 = np.asarray(inputs["hidden"], np.float32).reshape(H)
    attn_W = np.asarray(inputs["attn_W"], np.float32)
    attn_b = np.asarray(inputs["attn_b"], np.float32)
    b_ih = np.asarray(inputs["b_ih"], np.float32)
    b_hh = np.asarray(inputs["b_hh"], np.float32)

    attn_WT = np.ascontiguousarray(
        attn_W.T.reshape(K2, P, L).transpose(1, 0, 2))  # [128, 16, 15]
    comb_WT = np.ascontiguousarray(
        np.asarray(inputs["comb_W"], np.float32).T.astype(wnp))
    w_ihT = np.ascontiguousarray(
        np.asarray(inputs["w_ih"], np.float32).T.astype(wnp))
    w_hhT = np.ascontiguousarray(
        np.asarray(inputs["w_hh"], np.float32).T.astype(wnp))

    out_W = np.asarray(inputs["out_W"], np.float32)
    outWT_pad = np.zeros((H, vpad), wnp)
    outWT_pad[:, :V] = out_W.T.astype(wnp)
    outb_pad = np.full(vpad, PAD_BIAS, np.float32)
    outb_pad[:V] = np.asarray(inputs["out_b"], np.float32)

    common = dict(
        emb_kt=_kt(emb_row),
        h0_kt=_kt(h0),
        enc=np.ascontiguousarray(np.asarray(inputs["encoder_outputs"], np.float32)),
        attn_WT=attn_WT,
        attn_bT=np.ascontiguousarray(attn_b.reshape(L, 1)),
        attn_b=np.ascontiguousarray(attn_b.reshape(1, L)),
        comb_WT=comb_WT,
        w_ihT=w_ihT,
        w_hhT=w_hhT,
        b_comb=_kt(np.asarray(inputs["comb_b"], np.float32)),
        b_r=_kt(b_ih[0:H] + b_hh[0:H]),
        b_z=_kt(b_ih[H:2 * H] + b_hh[H:2 * H]),
        b_in=_kt(b_ih[2 * H:]),
        b_hn=_kt(b_hh[2 * H:]),
    )
    in_maps = []
    for c in range(NCORES):
        m = dict(common)
        m["outWT"] = np.ascontiguousarray(outWT_pad[:, c * vsh:(c + 1) * vsh])
        m["outb"] = np.ascontiguousarray(outb_pad[c * vsh:(c + 1) * vsh].reshape(1, vsh))
        in_maps.append(m)
    return in_maps


def assemble(results, nch=13):
    vsh = nch * CHUNK
    logits = np.concatenate(
        [results[c]["logits_s"].reshape(vsh) for c in range(NCORES)])[:V]
    total = np.float32(sum(float(np.asarray(results[c]["sumexp"]).reshape(-1)[0])
                           for c in range(NCORES)))
    out = (logits - np.float32(np.log(total))).reshape(1, V).astype(np.float32)
    h_new = np.ascontiguousarray(
        results[0]["h_new_kt"].T.reshape(1, 1, H).astype(np.float32))
    attn_w = np.ascontiguousarray(results[0]["attn_w"].reshape(1, L).astype(np.float32))
    return out, h_new, attn_w


def run(inputs, w_dtype="bfloat16", nch=13, trace=False):
    nc = _get_nc(w_dtype, nch)
    in_maps = make_in_maps(inputs, w_dtype, nch)
    res = run_bass_kernel_spmd(nc, in_maps, list(range(NCORES)), trace=trace)
    return assemble(res.results, nch), res


def kernel(**inputs):
    (out, h_new, attn_w), _ = run(inputs)
    return out, h_new, attn_w


# revision 20
# speedup vs baseline: 1.0033x; 1.0033x over previous
"""AttnDecoderRNN step on 8 Trainium2 NeuronCores.

Sharding: out_W column-parallel (vocab) across 8 cores; tiny attention+GRU
replicated on every core; embedding row gathered on host (4KB of emb_W);
log_softmax normalizer combined on host from per-core sum(exp(logits)).
"""

import numpy as np
from contextlib import ExitStack

import ml_dtypes

import concourse.bass as bass
import concourse.mybir as mybir
import concourse.tile as tile
from concourse import bacc
from concourse.bass_utils import run_bass_kernel_spmd

H = 1024
V = 50257
L = 15          # MAX_LEN
P = 128
KH = H // P     # 8 k-tiles per H-vector
K2 = 2 * H // P # 16 k-tiles per 2H-vector
M3 = 3 * H // P # 24 m-blocks of the 3H gate dim
NCORES = 8
CHUNK = 512

AF = mybir.ActivationFunctionType
F32 = mybir.dt.float32
PAD_BIAS = -10000.0  # exp() underflows to exactly 0.0


def _dt(name):
    return {"float32": F32, "bfloat16": mybir.dt.bfloat16,
            "float8e4": mybir.dt.float8e4}[name]


def _np_dt(name):
    return {"float32": np.float32, "bfloat16": ml_dtypes.bfloat16,
            "float8e4": ml_dtypes.float8_e4m3}[name]


def build_kernel(w_dtype="bfloat16", nch=13, num_cores=NCORES, out_dtype=None):
    """Build + compile the SPMD bass program. Returns compiled nc."""
    wdt = _dt(w_dtype)
    odt = _dt(out_dtype or w_dtype)
    vsh = nch * CHUNK  # padded vocab shard per core

    nc = bacc.Bacc(
        "TRN2",
        target_bir_lowering=False,
        debug=False,
        enable_asserts=True,
        num_devices=num_cores,
    )

    # ---- I/O ----
    emb_kt = nc.dram_tensor("emb_kt", [P, KH], F32, kind="ExternalInput").ap()
    h0_kt = nc.dram_tensor("h0_kt", [P, KH], F32, kind="ExternalInput").ap()
    enc = nc.dram_tensor("enc", [L, H], F32, kind="ExternalInput").ap()
    attn_WT = nc.dram_tensor("attn_WT", [P, K2, L], F32, kind="ExternalInput").ap()
    attn_bT = nc.dram_tensor("attn_bT", [L, 1], F32, kind="ExternalInput").ap()
    attn_b = nc.dram_tensor("attn_b", [1, L], F32, kind="ExternalInput").ap()
    comb_WT = nc.dram_tensor("comb_WT", [2 * H, H], wdt, kind="ExternalInput").ap()
    w_ihT = nc.dram_tensor("w_ihT", [H, 3 * H], wdt, kind="ExternalInput").ap()
    w_hhT = nc.dram_tensor("w_hhT", [H, 3 * H], wdt, kind="ExternalInput").ap()
    b_comb = nc.dram_tensor("b_comb", [P, KH], F32, kind="ExternalInput").ap()
    b_r = nc.dram_tensor("b_r", [P, KH], F32, kind="ExternalInput").ap()
    b_z = nc.dram_tensor("b_z", [P, KH], F32, kind="ExternalInput").ap()
    b_in = nc.dram_tensor("b_in", [P, KH], F32, kind="ExternalInput").ap()
    b_hn = nc.dram_tensor("b_hn", [P, KH], F32, kind="ExternalInput").ap()
    outWT = nc.dram_tensor("outWT", [H, vsh], odt, kind="ExternalInput").ap()
    outb = nc.dram_tensor("outb", [1, vsh], F32, kind="ExternalInput").ap()
    outs = nc.dram_tensor("outs", [1, vsh], F32, kind="ExternalInput").ap()

    logits_s = nc.dram_tensor("logits_s", [1, vsh], F32, kind="ExternalOutput").ap()
    sumexp = nc.dram_tensor("sumexp", [1, 1], F32, kind="ExternalOutput").ap()
    h_new_kt = nc.dram_tensor("h_new_kt", [P, KH], F32, kind="ExternalOutput").ap()
    attn_w_out = nc.dram_tensor("attn_w", [1, L], F32, kind="ExternalOutput").ap()

    with tile.TileContext(nc) as tc:
        with ExitStack() as ctx:
            _body(ctx, tc, wdt, odt, nch, vsh, locals())

    nc.compile()
    return nc


def _body(ctx, tc, wdt, odt, nch, vsh, t):
    nc = tc.nc
    fp32 = wdt == F32

    fp32_build = wdt == F32
    consts = ctx.enter_context(tc.tile_pool(name="consts", bufs=1))
    cpool = ctx.enter_context(tc.tile_pool(name="cpool", bufs=K2))
    wpool = ctx.enter_context(tc.tile_pool(name="wpool", bufs=8 if fp32_build else 12))
    # deep out_WT buffering: lets the logits weight stream prefetch during
    # the attention+GRU phase instead of serializing after it
    opool = ctx.enter_context(
        tc.tile_pool(name="opool", bufs=24 if fp32_build else 72))
    bpool = ctx.enter_context(tc.tile_pool(name="bpool", bufs=4))
    pp_s1 = ctx.enter_context(
        tc.tile_pool(name="pp_s1", bufs=2, space=bass.MemorySpace.PSUM))
    pp_s2 = ctx.enter_context(
        tc.tile_pool(name="pp_s2", bufs=1, space=bass.MemorySpace.PSUM))
    pp_gates = ctx.enter_context(
        tc.tile_pool(name="pp_gates", bufs=1, space=bass.MemorySpace.PSUM))
    pp_log = ctx.enter_context(
        tc.tile_pool(name="pp_log", bufs=3, space=bass.MemorySpace.PSUM))

    # ---- constant / small loads ----
    emb_t = consts.tile([P, KH], F32)
    nc.sync.dma_start(out=emb_t[:], in_=t["emb_kt"])
    h0_t = consts.tile([P, KH], F32)
    nc.sync.dma_start(out=h0_t[:], in_=t["h0_kt"])
    enc_t = consts.tile([L, H], F32)
    nc.sync.dma_start(out=enc_t[:], in_=t["enc"])
    attnW_t = consts.tile([P, K2, L], F32)
    nc.sync.dma_start(out=attnW_t[:], in_=t["attn_WT"])
    attn_bT_t = consts.tile([L, 1], F32)
    nc.sync.dma_start(out=attn_bT_t[:], in_=t["attn_bT"])
    attn_b_t = consts.tile([1, L], F32)
    nc.sync.dma_start(out=attn_b_t[:], in_=t["attn_b"])
    b_comb_t = consts.tile([P, KH], F32)
    nc.sync.dma_start(out=b_comb_t[:], in_=t["b_comb"])
    b_r_t = consts.tile([P, KH], F32)
    nc.sync.dma_start(out=b_r_t[:], in_=t["b_r"])
    b_z_t = consts.tile([P, KH], F32)
    nc.sync.dma_start(out=b_z_t[:], in_=t["b_z"])
    b_in_t = consts.tile([P, KH], F32)
    nc.sync.dma_start(out=b_in_t[:], in_=t["b_in"])
    b_hn_t = consts.tile([P, KH], F32)
    nc.sync.dma_start(out=b_hn_t[:], in_=t["b_hn"])


    ones_t = consts.tile([1, P], F32)
    nc.vector.memset(ones_t[:], 1.0)

    attn_in = consts.tile([P, K2], F32)
    nc.sync.dma_start(out=attn_in[:, 0:KH], in_=t["emb_kt"])
    nc.sync.dma_start(out=attn_in[:, KH:K2], in_=t["h0_kt"])
    comb_in = consts.tile([P, K2], F32)
    nc.sync.dma_start(out=comb_in[:, 0:KH], in_=t["emb_kt"])

    # ---- attention scores (both layouts) ----
    scores_ps = pp_s1.tile([1, L], F32, tag="s1")
    for k in range(K2):
        nc.tensor.matmul(scores_ps[:], attn_in[:, k:k + 1], attnW_t[:, k, :],
                         start=(k == 0), stop=(k == K2 - 1))
    scoresT_ps = pp_s2.tile([L, 1], F32, tag="s2")
    for k in range(K2):
        nc.tensor.matmul(scoresT_ps[:], attnW_t[:, k, :], attn_in[:, k:k + 1],
                         start=(k == 0), stop=(k == K2 - 1))

    # attn_weights output = softmax(scores + b); no max-subtraction (|s| ~ 1)
    scores_sb = consts.tile([1, L], F32)
    nc.vector.tensor_add(scores_sb[:], scores_ps[:], attn_b_t[:])
    exp_sb = consts.tile([1, L], F32)
    se_sb = consts.tile([1, 1], F32)
    nc.scalar.activation(exp_sb[:], scores_sb[:], AF.Exp, accum_out=se_sb[:])
    inv_se = consts.tile([1, 1], F32)
    nc.vector.reciprocal(inv_se[:], se_sb[:])
    attn_w_sb = consts.tile([1, L], F32)
    nc.vector.tensor_scalar_mul(attn_w_sb[:], exp_sb[:], inv_se[:])
    nc.sync.dma_start(out=t["attn_w_out"], in_=attn_w_sb[:])

    # transposed exp(scores) on 15 partitions for the applied matmul
    expT_sb = consts.tile([L, 1], F32)
    nc.scalar.activation(expT_sb[:], scoresT_ps[:], AF.Exp, bias=attn_bT_t[:])

    # attn_applied^T (unnormalized): [128, 8] blocks = enc^T @ expT
    aa_ps = pp_s1.tile([P, KH], F32, tag="s1")
    for m in range(KH):
        nc.tensor.matmul(aa_ps[:, m:m + 1], enc_t[:, m * P:(m + 1) * P],
                         expT_sb[:], start=True, stop=True)

    # broadcast sum(exp) to 128 partitions: ones^T @ se
    bc_ps = pp_s2.tile([P, 1], F32, tag="s2")
    nc.tensor.matmul(bc_ps[:], ones_t[:], se_sb[:], start=True, stop=True)
    inv_bc = consts.tile([P, 1], F32)
    nc.vector.reciprocal(inv_bc[:], bc_ps[:])
    nc.vector.tensor_scalar_mul(comb_in[:, KH:K2], aa_ps[:], inv_bc[:])

    comb_in_mm = comb_in
    if not fp32:
        comb_in_mm = consts.tile([P, K2], wdt)
        nc.vector.tensor_copy(comb_in_mm[:], comb_in[:])

    # ---- combine layer: x^T = comb_WT^T-blocks @ comb_in cols ----
    # m-outer / k-inner: each psum column's accumulation group completes
    # before the next starts (start=True re-arms the whole 2KB psum bank).
    x_ps = pp_s1.tile([P, KH], F32, tag="s1")
    cws = []
    for k in range(K2):
        cw = cpool.tile([P, H], wdt, tag="cw")
        nc.sync.dma_start(out=cw[:], in_=t["comb_WT"][k * P:(k + 1) * P, :])
        cws.append(cw)
    for m in range(KH):
        for k in range(K2):
            nc.tensor.matmul(x_ps[:, m:m + 1], cws[k][:, m * P:(m + 1) * P],
                             comb_in_mm[:, k:k + 1],
                             start=(k == 0), stop=(k == K2 - 1))
    x_t = consts.tile([P, KH], F32)
    nc.vector.tensor_add(x_t[:], x_ps[:], b_comb_t[:])
    nc.vector.tensor_scalar_max(x_t[:], x_t[:], 0.0)

    x_mm, h0_mm = x_t, h0_t
    if not fp32:
        x_mm = consts.tile([P, KH], wdt)
        nc.vector.tensor_copy(x_mm[:], x_t[:])
        h0_mm = consts.tile([P, KH], wdt)
        nc.vector.tensor_copy(h0_mm[:], h0_t[:])

    # ---- GRU gates: gi^T, gh^T as [128, 24] psum ----
    MC = 12          # m-blocks per chunk
    WCH = MC * P     # 1536 columns per resident chunk
    gi_ps = pp_gates.tile([P, M3], F32, tag="gi")
    gh_ps = pp_gates.tile([P, M3], F32, tag="gh")
    for src, vec, g_ps in ((t["w_ihT"], x_mm, gi_ps), (t["w_hhT"], h0_mm, gh_ps)):
        for mc in range(3 * H // WCH):
            wts = []
            for k in range(KH):
                wt = wpool.tile([P, WCH], wdt, tag="wt")
                nc.sync.dma_start(
                    out=wt[:],
                    in_=src[k * P:(k + 1) * P, mc * WCH:(mc + 1) * WCH])
                wts.append(wt)
            for m in range(MC):
                gm = mc * MC + m
                for k in range(KH):
                    nc.tensor.matmul(g_ps[:, gm:gm + 1],
                                     wts[k][:, m * P:(m + 1) * P],
                                     vec[:, k:k + 1],
                                     start=(k == 0), stop=(k == KH - 1))

    # ---- gates elementwise on [128, 8] ----
    # (VectorE may read at most one PSUM operand -> stage gi in SBUF)
    gi_sb = consts.tile([P, M3], F32)
    nc.scalar.activation(gi_sb[:], gi_ps[:], AF.Copy)
    r_t = consts.tile([P, KH], F32)
    nc.vector.tensor_add(r_t[:], gi_sb[:, 0:KH], gh_ps[:, 0:KH])
    nc.vector.tensor_add(r_t[:], r_t[:], b_r_t[:])
    nc.scalar.activation(r_t[:], r_t[:], AF.Sigmoid)
    z_t = consts.tile([P, KH], F32)
    nc.vector.tensor_add(z_t[:], gi_sb[:, KH:2 * KH], gh_ps[:, KH:2 * KH])
    nc.vector.tensor_add(z_t[:], z_t[:], b_z_t[:])
    nc.scalar.activation(z_t[:], z_t[:], AF.Sigmoid)
    hn_t = consts.tile([P, KH], F32)
    nc.vector.tensor_add(hn_t[:], gh_ps[:, 2 * KH:3 * KH], b_hn_t[:])
    n_t = consts.tile([P, KH], F32)
    nc.vector.tensor_add(n_t[:], gi_sb[:, 2 * KH:3 * KH], b_in_t[:])
    nc.vector.tensor_mul(hn_t[:], r_t[:], hn_t[:])
    nc.vector.tensor_add(n_t[:], n_t[:], hn_t[:])
    nc.scalar.activation(n_t[:], n_t[:], AF.Tanh)
    # h_new = n + z * (h0 - n)
    d_t = consts.tile([P, KH], F32)
    nc.vector.tensor_sub(d_t[:], h0_t[:], n_t[:])
    h_t = consts.tile([P, KH], F32)
    nc.vector.tensor_mul(h_t[:], z_t[:], d_t[:])
    nc.vector.tensor_add(h_t[:], h_t[:], n_t[:])
    nc.sync.dma_start(out=t["h_new_kt"], in_=h_t[:])

    h_mm = h_t
    if odt != F32:
        h_mm = consts.tile([P, KH], odt)
        nc.vector.tensor_copy(h_mm[:], h_t[:])

    # ---- output projection: stream out_WT shard ----
    scaled = odt not in (F32, mybir.dt.bfloat16)
    se_slots = consts.tile([1, nch], F32)
    exp_tmp = consts.tile([1, CHUNK], F32)
    for c in range(nch):
        csl = slice(c * CHUNK, (c + 1) * CHUNK)
        lg_ps = pp_log.tile([1, CHUNK], F32, tag="lg")
        for k in range(KH):
            ot = opool.tile([P, CHUNK], odt, tag="ot")
            nc.sync.dma_start(out=ot[:], in_=t["outWT"][k * P:(k + 1) * P, csl])
            nc.tensor.matmul(lg_ps[:], h_mm[:, k:k + 1], ot[:],
                             start=(k == 0), stop=(k == KH - 1))
        ob = bpool.tile([1, CHUNK], F32, tag="ob")
        nc.sync.dma_start(out=ob[:], in_=t["outb"][:, csl])
        lg = bpool.tile([1, CHUNK], F32, tag="lgout")
        if scaled:
            sc = bpool.tile([1, CHUNK], F32, tag="sc")
            nc.sync.dma_start(out=sc[:], in_=t["outs"][:, csl])
            nc.vector.tensor_mul(lg[:], lg_ps[:], sc[:])
            nc.vector.tensor_add(lg[:], lg[:], ob[:])
        else:
            nc.vector.tensor_add(lg[:], lg_ps[:], ob[:])
        nc.scalar.activation(exp_tmp[:], lg[:], AF.Exp,
                             accum_out=se_slots[:, c:c + 1])
        nc.sync.dma_start(out=t["logits_s"][:, csl], in_=lg[:])
    se_total = consts.tile([1, 1], F32)
    nc.vector.reduce_sum(se_total[:], se_slots[:], axis=mybir.AxisListType.X)
    nc.sync.dma_start(out=t["sumexp"], in_=se_total[:])


# ---------------------------------------------------------------------------
# host side
# ---------------------------------------------------------------------------

_CACHE = {}


def _get_nc(w_dtype, nch, out_dtype=None):
    key = (w_dtype, nch, out_dtype or w_dtype)
    if key not in _CACHE:
        _CACHE[key] = build_kernel(w_dtype, nch, out_dtype=out_dtype)
    return _CACHE[key]


def _kt(v):
    """[H*k] vector -> [128, k] partition-major tile layout."""
    v = np.asarray(v, np.float32)
    return np.ascontiguousarray(v.reshape(-1, P).T)


def make_in_maps(inputs, w_dtype="bfloat16", nch=13, out_dtype=None):
    wnp = _np_dt(w_dtype)
    out_dtype = out_dtype or w_dtype
    onp = _np_dt(out_dtype)
    vsh = nch * CHUNK
    vpad = vsh * NCORES

    idx = int(np.asarray(inputs["input_seq"]).reshape(-1)[0])
    emb_row = np.asarray(inputs["emb_W"], np.float32)[idx]
    h0 = np.asarray(inputs["hidden"], np.float32).reshape(H)
    attn_W = np.asarray(inputs["attn_W"], np.float32)
    attn_b = np.asarray(inputs["attn_b"], np.float32)
    b_ih = np.asarray(inputs["b_ih"], np.float32)
    b_hh = np.asarray(inputs["b_hh"], np.float32)

    attn_WT = np.ascontiguousarray(
        attn_W.T.reshape(K2, P, L).transpose(1, 0, 2))  # [128, 16, 15]
    comb_WT = np.ascontiguousarray(
        np.asarray(inputs["comb_W"], np.float32).T.astype(wnp))
    w_ihT = np.ascontiguousarray(
        np.asarray(inputs["w_ih"], np.float32).T.astype(wnp))
    w_hhT = np.ascontiguousarray(
        np.asarray(inputs["w_hh"], np.float32).T.astype(wnp))

    out_W = np.asarray(inputs["out_W"], np.float32)
    outs_pad = np.ones(vpad, np.float32)
    if out_dtype == "float8e4":
        # per-vocab-row scales so fp8 E4M3 (TRN max ±240) covers each row
        s = np.abs(out_W).max(axis=1) / 224.0
        s = np.maximum(s, 1e-30)
        out_Wq = np.clip(out_W / s[:, None], -240.0, 240.0)
        outs_pad[:V] = s
        outWT_pad = np.zeros((H, vpad), onp)
        outWT_pad[:, :V] = out_Wq.T.astype(onp)
    else:
        outWT_pad = np.zeros((H, vpad), onp)
        outWT_pad[:, :V] = out_W.T.astype(onp)
    outb_pad = np.full(vpad, PAD_BIAS, np.float32)
    outb_pad[:V] = np.asarray(inputs["out_b"], np.float32)

    common = dict(
        emb_kt=_kt(emb_row),
        h0_kt=_kt(h0),
        enc=np.ascontiguousarray(np.asarray(inputs["encoder_outputs"], np.float32)),
        attn_WT=attn_WT,
        attn_bT=np.ascontiguousarray(attn_b.reshape(L, 1)),
        attn_b=np.ascontiguousarray(attn_b.reshape(1, L)),
        comb_WT=comb_WT,
        w_ihT=w_ihT,
        w_hhT=w_hhT,
        b_comb=_kt(np.asarray(inputs["comb_b"], np.float32)),
        b_r=_kt(b_ih[0:H] + b_hh[0:H]),
        b_z=_kt(b_ih[H:2 * H] + b_hh[H:2 * H]),
        b_in=_kt(b_ih[2 * H:]),
        b_hn=_kt(b_hh[2 * H:]),
    )
    in_maps = []
    for c in range(NCORES):
        m = dict(common)
        m["outWT"] = np.ascontiguousarray(outWT_pad[:, c * vsh:(c + 1) * vsh])
        m["outb"] = np.ascontiguousarray(outb_pad[c * vsh:(c + 1) * vsh].reshape(1, vsh))
        m["outs"] = np.ascontiguousarray(outs_pad[c * vsh:(c + 1) * vsh].reshape(1, vsh))
        in_maps.append(m)
    return in_maps


def assemble(results, nch=13):
    vsh = nch * CHUNK
    logits = np.concatenate(
        [results[c]["logits_s"].reshape(vsh) for c in range(NCORES)])[:V]
    total = np.float32(sum(float(np.asarray(results[c]["sumexp"]).reshape(-1)[0])
                           for c in range(NCORES)))
    out = (logits - np.float32(np.log(total))).reshape(1, V).astype(np.float32)
    h_new = np.ascontiguousarray(
        results[0]["h_new_kt"].T.reshape(1, 1, H).astype(np.float32))
    attn_w = np.ascontiguousarray(results[0]["attn_w"].reshape(1, L).astype(np.float32))
    return out, h_new, attn_w


def run(inputs, w_dtype="bfloat16", nch=13, trace=False, out_dtype=None):
    nc = _get_nc(w_dtype, nch, out_dtype)
    in_maps = make_in_maps(inputs, w_dtype, nch, out_dtype)
    res = run_bass_kernel_spmd(nc, in_maps, list(range(NCORES)), trace=trace)
    return assemble(res.results, nch), res


def kernel(**inputs):
    (out, h_new, attn_w), _ = run(inputs)
    return out, h_new, attn_w
